# revision 37
# baseline (speedup 1.0000x reference)
"""Linear-attention head (elu+1 feature map) on 8 TRN2 NeuronCores.

Pure data parallel: batch 16 -> 2 batches per core. Sparse-attention
compaction: the padding mask zeroes rows of phi_q/phi_k/v, and (because
S == DH) the same mask thins the O-GEMM contraction. All sequence dims
are host-compacted to C = max kept count (~531 of 1024), and the three
projection weights are row-permuted per batch to kept-first order so the
q-feature axis aligns with the compacted v-row axis of A:

  perm      = [kept_positions..., padded_positions...]
  xt[m,j]   = x[idx[j], m]                      (j < n, zero-padded to C)
  kt[d',j]  = phi_k^T in perm feature order     (pad cols forced to 0 via
                                                 rank-1 -1e9 row in PSUM)
  vt[d',j]  = (Wv_perm x + bv)*keep             (pad cols zeroed)
  qt[d',s]  = phi_q^T (pad s cols garbage; host discards)
  A[i,j]    = sum_d' vt[d',i]*kt[d',j]          (rows i>=n exactly 0)
  O[s,t]    = sum_{d'<C} qt[d',s]*A[d',t]       (exact: A rows >= n vanish)
  den[s]    = sum_{all d'} qt[d',s]*ksum[d']    (ksum = free-dim accum of kt)
  out       = O / max(den, eps), host-scattered into the full [S,S] grid.

All matmuls run in bf16 (1 cycle/row at any N, FWL weight loads); PSUM
accumulation is fp32. elu(x)+1 is computed exactly as min(exp(x),1)+relu(x).

Host-side work is layout marshalling only (mask indexing / transposes /
dtype casts).
"""

import sys

import numpy as np

if "/opt/trn_rl_repo" not in sys.path:
    sys.path.insert(0, "/opt/trn_rl_repo")

B, S, DM, DH = 16, 1024, 1024, 1024
NCORES = 8
BPC = B // NCORES  # batches per core
P = 128
NT = DM // P  # 8 tiles of 128 along the model/feature dims
NEG = -1.0e9
EPS = 1e-6

_CACHE = {}


def _build_nc(C):
    import concourse.bacc as bacc
    import concourse.bass as bass
    import concourse.mybir as mybir
    import concourse.tile as tile

    f32 = mybir.dt.float32
    bf16 = mybir.dt.bfloat16
    Act = mybir.ActivationFunctionType
    Op = mybir.AluOpType

    NTC = (C + P - 1) // P  # partition tiles over the compacted dim
    rows = [min(P, C - i * P) for i in range(NTC)]
    if C > 512:
        chunks = [(0, 512), (512, C)]
    else:
        chunks = [(0, C)]

    nc = bacc.Bacc()

    # partition-major DRAM layouts: per-partition runs of NT*C (x), NT*DH
    # (weights) and NTC*C (out) elements give multi-KB DMA packets instead
    # of per-row ~1KB ones.
    xt_ext = nc.declare_dram_parameter("xt", [BPC, P, NT * C], bf16, isOutput=False)
    wt_ext = nc.declare_dram_parameter("wt", [BPC, 3, P, NT * DH], bf16, isOutput=False)
    bias_ext = nc.declare_dram_parameter("bias", [BPC, P, 3 * NT], f32, isOutput=False)
    mrow_ext = nc.declare_dram_parameter("mrow", [BPC, C], bf16, isOutput=False)
    consts_ext = nc.declare_dram_parameter("consts", [2, P], bf16, isOutput=False)
    out_ext = nc.declare_dram_parameter("out", [BPC, P, NTC * C], bf16, isOutput=True)

    W_Q, W_K, W_V = 0, 1, 2
    BIAS_COL = {W_Q: 0, W_K: NT, W_V: 2 * NT}

    with tile.TileContext(nc) as tc:
        with (
            tc.tile_pool(name="const", bufs=1) as cpool,
            tc.tile_pool(name="rows", bufs=2) as rpool,
            tc.tile_pool(name="keept", bufs=2) as ktpool,
            tc.tile_pool(name="tiny", bufs=3) as spool,
            tc.tile_pool(name="xt", bufs=2) as xtpool,
            tc.tile_pool(name="at", bufs=NTC + 2) as atpool,
            tc.tile_pool(name="kvq", bufs=NT + 1) as kvqpool,
            tc.tile_pool(name="wt", bufs=6) as wpool,
            tc.tile_pool(name="actE", bufs=2) as apool,
            tc.tile_pool(name="actR", bufs=2) as rrpool,
            tc.tile_pool(name="ost", bufs=2) as opool,
            tc.tile_pool(name="ps", bufs=3, space="PSUM") as pspool,
            tc.tile_pool(name="psden", bufs=2, space="PSUM") as dpool,
        ):
            # ---- constants ----
            ones_col = cpool.tile([1, P], bf16, tag="ones")
            nc.sync.dma_start(ones_col[:], consts_ext[0:1, :])
            neg_col = cpool.tile([1, P], bf16, tag="neg")
            nc.sync.dma_start(neg_col[:], consts_ext[1:2, :])

            def fence(reads, writes):
                # walrus' Matmult pseudo carries at most ONE embedded sync
                # wait. A PE NoOp declaring the group's reads/writes absorbs
                # all foreign-proc waits (NoOp carries many, like the Tile
                # tail drain), leaving each matmul's own wait count <= 1.
                eng = nc.tensor
                eng.add_instruction(
                    mybir.InstNoOp(
                        name=nc.get_next_instruction_name(),
                        text_hint="dep_fence",
                        bass_nofuse=True,
                        ins=[eng.lower_ap(a) for a in reads],
                        outs=[eng.lower_ap(a) for a in writes],
                    )
                )

            def mm_psum(reads):
                ps = pspool.tile([P, C], f32, tag="mm")
                fence(reads, [ps[:]])
                return ps

            # ---- prologue: allocate input tiles (full double buffering, no
            # ring reuse => no WAR waits) and issue only the first batch's
            # critical DMAs. The rest are paced one phase ahead of use via
            # `hooks` — front-loading everything triggers the chip's P0
            # power downclock (everything runs ~1.2x slower). ----
            pre = []
            for b in range(BPC):
                d = {}
                d["mrow"] = rpool.tile([1, C], bf16, tag="mrow", name="mrow")
                d["bias"] = spool.tile([P, 3 * NT], f32, tag="bias", name="bias_sb")
                d["xt"] = xtpool.tile([P, NT * C], bf16, tag="xt", name="xta")
                d["wt"] = {
                    which: wpool.tile([P, NT * DH], bf16, tag="wt", name="wta")
                    for which in (1, 2, 0)  # consumption order K, V, Q
                }
                pre.append(d)

            def dma_xt(b):
                xta = pre[b]["xt"]
                Q = NT * C // 4
                for q in range(4):
                    nc.sync.dma_start(
                        xta[:, q * Q : (q + 1) * Q],
                        xt_ext[b, :, q * Q : (q + 1) * Q],
                    )

            def dma_wt(b, which):
                wt = pre[b]["wt"][which]
                Q = NT * DH // 4
                for q in range(4):
                    nc.gpsimd.dma_start(
                        wt[:, q * Q : (q + 1) * Q],
                        wt_ext[b, which, :, q * Q : (q + 1) * Q],
                    )

            # x + K weights first — they gate the first projection; the
            # small mask/bias transfers queue behind them.
            dma_xt(0)
            dma_wt(0, 1)
            for b in range(BPC):
                nc.sync.dma_start(pre[b]["mrow"][:], mrow_ext[b : b + 1, :])
                nc.sync.dma_start(pre[b]["bias"][:], bias_ext[b])
            hooks = [
                lambda: dma_wt(0, 2),   # at b0 K proj: prefetch b0 V weights
                lambda: dma_wt(0, 0),   # at b0 V proj: prefetch b0 Q weights
                lambda: dma_xt(1),      # at b0 Q proj: prefetch b1 x
                lambda: dma_wt(1, 1),   # at b0 A:      prefetch b1 K weights
                lambda: dma_wt(1, 2),   # at b0 den:    prefetch b1 V weights
                lambda: dma_wt(1, 0),   # at b0 O:      prefetch b1 Q weights
            ]

            def next_hook():
                if hooks:
                    hooks.pop(0)()

            for b in range(BPC):
                mrow = pre[b]["mrow"]
                bias_sb = pre[b]["bias"]
                xt = pre[b]["xt"]

                # ---- projections ----
                def project(which, masked_rank1):
                    """Yields NT [128(d'), C(s)] PSUM tiles, one per dt."""
                    wta = pre[b]["wt"][which]
                    tiles = []
                    for dt in range(NT):
                        # the weight tile is deliberately NOT in the fence:
                        # each matmul carries its own single DMA wait, so
                        # the stream starts as soon as x + weights land.
                        deps = [xt[:, : NT * C // 4]]
                        if masked_rank1:
                            deps += [neg_col[:], mrow[:]]
                        ps = mm_psum(deps)
                        for mt in range(NT):
                            ws = slice(mt * DH + dt * P, mt * DH + (dt + 1) * P)
                            for c0, c1 in chunks:
                                nc.tensor.matmul(
                                    ps[:, c0:c1],
                                    wta[:, ws],
                                    xt[:, mt * C + c0 : mt * C + c1],
                                    start=(mt == 0),
                                    stop=(mt == NT - 1) and not masked_rank1,
                                )
                        if masked_rank1:
                            for c0, c1 in chunks:
                                nc.tensor.matmul(
                                    ps[:, c0:c1], neg_col[:], mrow[:, c0:c1],
                                    start=False, stop=(c1 == chunks[-1][1]),
                                )
                        tiles.append(ps)
                    return tiles

                # K projection (perm feature order): rank-1 -1e9*pad row
                # forces phi_k at tail cols to 0
                next_hook()
                kt = []
                ksum = spool.tile([P, NT + 1], bf16, tag="ksum")
                for dt, ps in enumerate(project(W_K, True)):
                    bcol = bias_sb[:, BIAS_COL[W_K] + dt : BIAS_COL[W_K] + dt + 1]
                    E = apool.tile([P, C], f32, tag="E")
                    nc.scalar.activation(E[:], ps[:], Act.Exp, bias=bcol)
                    R = rrpool.tile([P, C], f32, tag="R")
                    nc.scalar.activation(R[:], ps[:], Act.Relu, bias=bcol)
                    t = kvqpool.tile([P, C], bf16, tag="kt")
                    nc.vector.scalar_tensor_tensor(
                        out=t[:], in0=E[:], scalar=1.0, in1=R[:],
                        op0=Op.min, op1=Op.add,
                        accum_out=ksum[:, dt : dt + 1],
                    )
                    kt.append(t)

                # ---- pad-mask keep tile (1 - pad broadcast): emitted here,
                # after the K projection, so the PE's first work needs only
                # x + K weights ----
                kb_ps = mm_psum([ones_col[:], mrow[:]])
                for c0, c1 in chunks:
                    nc.tensor.matmul(
                        kb_ps[:, c0:c1], ones_col[:], mrow[:, c0:c1],
                        start=True, stop=True,
                    )
                keep_tile = ktpool.tile([P, C], f32, tag="keeptile")
                nc.vector.tensor_scalar(
                    out=keep_tile[:], in0=kb_ps[:], scalar1=-1.0, scalar2=1.0,
                    op0=Op.mult, op1=Op.add,
                )

                # V projection: (psum + bv) * keep  (zeroes tail cols)
                next_hook()
                vt = []
                for dt, ps in enumerate(project(W_V, False)):
                    bcol = bias_sb[:, BIAS_COL[W_V] + dt : BIAS_COL[W_V] + dt + 1]
                    t = kvqpool.tile([P, C], bf16, tag="vt")
                    nc.vector.scalar_tensor_tensor(
                        out=t[:], in0=ps[:], scalar=bcol, in1=keep_tile[:],
                        op0=Op.add, op1=Op.mult,
                    )
                    vt.append(t)

                # Q projection: unmasked phi_q (tail s cols discarded on host)
                next_hook()
                qt = []
                for dt, ps in enumerate(project(W_Q, False)):
                    bcol = bias_sb[:, BIAS_COL[W_Q] + dt : BIAS_COL[W_Q] + dt + 1]
                    E = apool.tile([P, C], f32, tag="E")
                    nc.scalar.activation(E[:], ps[:], Act.Exp, bias=bcol)
                    R = rrpool.tile([P, C], f32, tag="R")
                    nc.scalar.activation(R[:], ps[:], Act.Relu, bias=bcol)
                    t = kvqpool.tile([P, C], bf16, tag="qt")
                    nc.vector.scalar_tensor_tensor(
                        out=t[:], in0=E[:], scalar=1.0, in1=R[:],
                        op0=Op.min, op1=Op.add,
                        # the den matmuls run at N=2 with a pad column of
                        # ksum that must hold real data — fill it with a
                        # q-side accum.
                        accum_out=(
                            ksum[:, NT : NT + 1] if dt == NT - 1 else None
                        ),
                    )
                    qt.append(t)

                # ---- A = V @ phi_k^T  (A[i,j], i=v col (compact), j=k col) ----
                next_hook()
                at = []
                for it in range(NTC):
                    ri = rows[it]
                    isl = slice(it * P, it * P + ri)
                    ps = mm_psum([t[:] for t in vt] + [t[:] for t in kt])
                    for dt in range(NT):
                        for c0, c1 in chunks:
                            nc.tensor.matmul(
                                ps[:ri, c0:c1],
                                vt[dt][:, isl],
                                kt[dt][:, c0:c1],
                                start=(dt == 0), stop=(dt == NT - 1),
                            )
                    t = atpool.tile([P, C], bf16, tag="at")
                    nc.vector.tensor_copy(t[:ri, :], ps[:ri, :])
                    at.append(t)

                # ---- denominator (only needs qt + ksum; runs while the at
                # evacuations drain so the O loop ships output immediately) ----
                next_hook()
                zs = []
                for st in range(NTC):
                    rs = rows[st]
                    ss = slice(st * P, st * P + rs)
                    dps = dpool.tile([P, 2], f32, tag="den")
                    fence([t[:] for t in qt] + [ksum[:]], [dps[:]])
                    for dt in range(NT):
                        nc.tensor.matmul(
                            dps[:rs, :],
                            qt[dt][:, ss],
                            ksum[:, dt : dt + 2],
                            start=(dt == 0), stop=(dt == NT - 1),
                        )
                    dsb = spool.tile([P, 1], f32, tag="dsb")
                    nc.vector.tensor_scalar(
                        out=dsb[:rs], in0=dps[:rs, 0:1], scalar1=float(EPS),
                        scalar2=None, op0=Op.max,
                    )
                    z = spool.tile([P, 1], f32, tag="z", bufs=NTC + 1)
                    nc.vector.reciprocal(z[:rs], dsb[:rs])
                    zs.append(z)

                # ---- O = phi_q[:, :C] @ A, scale, single batched store ----
                next_hook()
                o = opool.tile([P, NTC * C], bf16, tag="ost")
                for st in range(NTC):
                    rs = rows[st]
                    ss = slice(st * P, st * P + rs)
                    ps = pspool.tile([P, C], f32, tag="mm")
                    fence([t[:] for t in qt] + [t[:] for t in at], [ps[:]])
                    for it in range(NTC):
                        ri = rows[it]
                        for c0, c1 in chunks:
                            nc.tensor.matmul(
                                ps[:rs, c0:c1],
                                qt[it][:ri, ss],
                                at[it][:ri, c0:c1],
                                start=(it == 0), stop=(it == NTC - 1),
                            )
                    nc.vector.tensor_scalar(
                        out=o[:rs, st * C : (st + 1) * C], in0=ps[:rs, :],
                        scalar1=zs[st][:rs], scalar2=None, op0=Op.mult,
                    )
                    # ship full-row blocks as they complete so the store
                    # overlaps the remaining O groups
                    if rs == P and (st % 2 == 1 or st == NTC - 1):
                        lo = (st // 2) * 2 * C
                        nc.sync.dma_start(
                            out_ext[b, :, lo : (st + 1) * C],
                            o[:, lo : (st + 1) * C],
                        )
                    elif rs < P:
                        if st % 2 == 1:
                            lo = (st // 2) * 2 * C
                            nc.sync.dma_start(
                                out_ext[b, :, lo : st * C], o[:, lo : st * C]
                            )
                        nc.sync.dma_start(
                            out_ext[b, :rs, st * C : (st + 1) * C],
                            o[:rs, st * C : (st + 1) * C],
                        )

    nc.compile()
    return nc


def _prepare_in_maps(inputs):
    import concourse.mybir as mybir

    npbf16 = mybir.dt.np(mybir.dt.bfloat16)

    x = np.asarray(inputs["x"], np.float32)
    pm = np.asarray(inputs["padding_mask"])
    W = [np.asarray(inputs[k], np.float32) for k in ("Wq", "Wk", "Wv")]
    bias = [np.asarray(inputs[k], np.float32) for k in ("bq", "bk", "bv")]

    idx_list = [np.nonzero(pm[b] != 1)[0] for b in range(B)]
    ns = [len(i) for i in idx_list]
    C = max(max(ns), 2)

    # partition-major device layouts (see _build_nc)
    xt = np.zeros((B, P, NT, C), npbf16)
    wt = np.zeros((B, 3, P, NT, DH), npbf16)
    bias_t = np.zeros((B, P, 3 * NT), np.float32)
    mrow = np.zeros((B, C), npbf16)
    for b in range(B):
        idx = idx_list[b]
        n = ns[b]
        rest = np.nonzero(pm[b] == 1)[0]
        perm = np.concatenate([idx, rest])
        # [p, mt, c] = x[idx[c], mt*P+p]
        xc = x[b, idx, :].T.reshape(NT, P, n).transpose(1, 0, 2)
        xt[b, :, :, :n] = xc.astype(npbf16)
        mrow[b, n:] = 1.0
        for w in range(3):
            wp = W[w][perm]  # [DH(d' perm), DM(m)]
            # [p, mt, d] = wp[d, mt*P+p]
            wt[b, w] = wp.T.reshape(NT, P, DH).transpose(1, 0, 2).astype(npbf16)
            bias_t[b, :, w * NT : (w + 1) * NT] = bias[w][perm].reshape(NT, P).T

    consts = np.stack(
        [np.ones(P, np.float32), np.full(P, NEG, np.float32)]
    ).astype(npbf16)

    in_maps = []
    for i in range(NCORES):
        sl = slice(BPC * i, BPC * (i + 1))
        in_maps.append(
            {
                "xt": np.ascontiguousarray(xt[sl]).reshape(BPC, P, NT * C),
                "wt": np.ascontiguousarray(wt[sl]).reshape(BPC, 3, P, NT * DH),
                "bias": np.ascontiguousarray(bias_t[sl]),
                "mrow": np.ascontiguousarray(mrow[sl]),
                "consts": consts,
            }
        )
    return C, ns, idx_list, in_maps


def _run(inputs, **kw):
    from concourse.bass_utils import run_bass_kernel_spmd

    C, ns, idx_list, in_maps = _prepare_in_maps(inputs)
    if C not in _CACHE:
        _CACHE[C] = _build_nc(C)
    nc = _CACHE[C]
    res = run_bass_kernel_spmd(nc, in_maps, core_ids=list(range(NCORES)), **kw)
    NTC = (C + P - 1) // P
    out = np.zeros((B, S, S), np.float32)
    for b in range(B):
        core, off = divmod(b, BPC)
        n = ns[b]
        idx = idx_list[b]
        # [P, NTC, C] -> [NTC*P, C] row-major over s
        oc = (
            np.asarray(res.results[core]["out"])[off]
            .reshape(P, NTC, C)
            .transpose(1, 0, 2)
            .reshape(NTC * P, C)
            .astype(np.float32)
        )
        out[b][np.ix_(idx, idx)] = oc[:n, :n]
    return out, res


def kernel(**inputs):
    out, _ = _run(inputs)
    return out


# revision 38
# speedup vs baseline: 1.0215x; 1.0215x over previous
"""Linear-attention head (elu+1 feature map) on 8 TRN2 NeuronCores.

Pure data parallel: batch 16 -> 2 batches per core. Sparse-attention
compaction: the padding mask zeroes rows of phi_q/phi_k/v, and (because
S == DH) the same mask thins the O-GEMM contraction. All sequence dims
are host-compacted to C = max kept count (~531 of 1024), and the three
projection weights are row-permuted per batch to kept-first order so the
q-feature axis aligns with the compacted v-row axis of A:

  perm      = [kept_positions..., padded_positions...]
  xt[m,j]   = x[idx[j], m]                      (j < n, zero-padded to C)
  kt[d',j]  = phi_k^T in perm feature order     (pad cols forced to 0 via
                                                 rank-1 -1e9 row in PSUM)
  vt[d',j]  = (Wv_perm x + bv)*keep             (pad cols zeroed)
  qt[d',s]  = phi_q^T (pad s cols garbage; host discards)
  A[i,j]    = sum_d' vt[d',i]*kt[d',j]          (rows i>=n exactly 0)
  O[s,t]    = sum_{d'<C} qt[d',s]*A[d',t]       (exact: A rows >= n vanish)
  den[s]    = sum_{all d'} qt[d',s]*ksum[d']    (ksum = free-dim accum of kt)
  out       = O / max(den, eps), host-scattered into the full [S,S] grid.

All matmuls run in bf16 (1 cycle/row at any N, FWL weight loads); PSUM
accumulation is fp32. elu(x)+1 is computed exactly as min(exp(x),1)+relu(x).

Host-side work is layout marshalling only (mask indexing / transposes /
dtype casts).
"""

import sys

import numpy as np

if "/opt/trn_rl_repo" not in sys.path:
    sys.path.insert(0, "/opt/trn_rl_repo")

B, S, DM, DH = 16, 1024, 1024, 1024
NCORES = 8
BPC = B // NCORES  # batches per core
P = 128
NT = DM // P  # 8 tiles of 128 along the model/feature dims
NEG = -1.0e9
EPS = 1e-6

_CACHE = {}


def _build_nc(C):
    import concourse.bacc as bacc
    import concourse.bass as bass
    import concourse.mybir as mybir
    import concourse.tile as tile

    f32 = mybir.dt.float32
    bf16 = mybir.dt.bfloat16
    Act = mybir.ActivationFunctionType
    Op = mybir.AluOpType

    NTC = (C + P - 1) // P  # partition tiles over the compacted dim
    rows = [min(P, C - i * P) for i in range(NTC)]
    if C > 512:
        chunks = [(0, 512), (512, C)]
    else:
        chunks = [(0, C)]

    nc = bacc.Bacc()

    # partition-major DRAM layouts: per-partition runs of NT*C (x), NT*DH
    # (weights) and NTC*C (out) elements give multi-KB DMA packets instead
    # of per-row ~1KB ones.
    xt_ext = nc.declare_dram_parameter("xt", [BPC, P, NT * C], bf16, isOutput=False)
    wt_ext = nc.declare_dram_parameter("wt", [BPC, 3, P, NT * DH], bf16, isOutput=False)
    bias_ext = nc.declare_dram_parameter("bias", [BPC, P, 3 * NT], f32, isOutput=False)
    mrow_ext = nc.declare_dram_parameter("mrow", [BPC, C], bf16, isOutput=False)
    consts_ext = nc.declare_dram_parameter("consts", [2, P], bf16, isOutput=False)
    out_ext = nc.declare_dram_parameter("out", [BPC, P, NTC * C], bf16, isOutput=True)

    W_Q, W_K, W_V = 0, 1, 2
    BIAS_COL = {W_Q: 0, W_K: NT, W_V: 2 * NT}

    with tile.TileContext(nc) as tc:
        with (
            tc.tile_pool(name="const", bufs=1) as cpool,
            tc.tile_pool(name="rows", bufs=2) as rpool,
            tc.tile_pool(name="keept", bufs=2) as ktpool,
            tc.tile_pool(name="tiny", bufs=3) as spool,
            tc.tile_pool(name="xt", bufs=2) as xtpool,
            tc.tile_pool(name="at", bufs=NTC + 2) as atpool,
            tc.tile_pool(name="kvq", bufs=NT + 1) as kvqpool,
            tc.tile_pool(name="wt", bufs=6) as wpool,
            tc.tile_pool(name="actE", bufs=2) as apool,
            tc.tile_pool(name="actR", bufs=2) as rrpool,
            tc.tile_pool(name="ost", bufs=2) as opool,
            tc.tile_pool(name="ps", bufs=3, space="PSUM") as pspool,
            tc.tile_pool(name="psden", bufs=2, space="PSUM") as dpool,
        ):
            # ---- constants ----
            ones_col = cpool.tile([1, P], bf16, tag="ones")
            nc.sync.dma_start(ones_col[:], consts_ext[0:1, :])
            neg_col = cpool.tile([1, P], bf16, tag="neg")
            nc.sync.dma_start(neg_col[:], consts_ext[1:2, :])

            def fence(reads, writes):
                # walrus' Matmult pseudo carries at most ONE embedded sync
                # wait. A PE NoOp declaring the group's reads/writes absorbs
                # all foreign-proc waits (NoOp carries many, like the Tile
                # tail drain), leaving each matmul's own wait count <= 1.
                eng = nc.tensor
                eng.add_instruction(
                    mybir.InstNoOp(
                        name=nc.get_next_instruction_name(),
                        text_hint="dep_fence",
                        bass_nofuse=True,
                        ins=[eng.lower_ap(a) for a in reads],
                        outs=[eng.lower_ap(a) for a in writes],
                    )
                )

            def mm_psum(reads):
                ps = pspool.tile([P, C], f32, tag="mm")
                fence(reads, [ps[:]])
                return ps

            # ---- prologue: allocate input tiles (full double buffering, no
            # ring reuse => no WAR waits) and issue only the first batch's
            # critical DMAs. The rest are paced one phase ahead of use via
            # `hooks` — front-loading everything triggers the chip's P0
            # power downclock (everything runs ~1.2x slower). ----
            pre = []
            for b in range(BPC):
                d = {}
                d["mrow"] = rpool.tile([1, C], bf16, tag="mrow", name="mrow")
                d["bias"] = spool.tile([P, 3 * NT], f32, tag="bias", name="bias_sb")
                d["xt"] = xtpool.tile([P, NT * C], bf16, tag="xt", name="xta")
                d["wt"] = {
                    which: wpool.tile([P, NT * DH], bf16, tag="wt", name="wta")
                    for which in (1, 2, 0)  # consumption order K, V, Q
                }
                pre.append(d)

            def dma_xt(b):
                xta = pre[b]["xt"]
                Q = NT * C // 4
                for q in range(4):
                    nc.sync.dma_start(
                        xta[:, q * Q : (q + 1) * Q],
                        xt_ext[b, :, q * Q : (q + 1) * Q],
                    )

            def dma_wt(b, which):
                # 8 chunks: fills the gpsimd queue's 8 DMA semaphores, so a
                # later projection's chunks wait for recycling instead of
                # diluting the bandwidth of the transfers that gate compute
                wt = pre[b]["wt"][which]
                Q = NT * DH // 8
                for q in range(8):
                    nc.gpsimd.dma_start(
                        wt[:, q * Q : (q + 1) * Q],
                        wt_ext[b, which, :, q * Q : (q + 1) * Q],
                    )

            # x + K weights first — they gate the first projection; the
            # small mask/bias transfers queue behind them.
            dma_xt(0)
            dma_wt(0, 1)
            for b in range(BPC):
                nc.sync.dma_start(pre[b]["mrow"][:], mrow_ext[b : b + 1, :])
                nc.sync.dma_start(pre[b]["bias"][:], bias_ext[b])
            hooks = [
                lambda: dma_wt(0, 2),   # at b0 K proj: prefetch b0 V weights
                lambda: dma_wt(0, 0),   # at b0 V proj: prefetch b0 Q weights
                lambda: dma_xt(1),      # at b0 Q proj: prefetch b1 x
                lambda: dma_wt(1, 1),   # at b0 A:      prefetch b1 K weights
                lambda: dma_wt(1, 2),   # at b0 den:    prefetch b1 V weights
                lambda: dma_wt(1, 0),   # at b0 O:      prefetch b1 Q weights
            ]

            def next_hook():
                if hooks:
                    hooks.pop(0)()

            for b in range(BPC):
                mrow = pre[b]["mrow"]
                bias_sb = pre[b]["bias"]
                xt = pre[b]["xt"]

                # ---- projections ----
                def project(which, masked_rank1):
                    """Yields NT [128(d'), C(s)] PSUM tiles, one per dt."""
                    wta = pre[b]["wt"][which]
                    tiles = []
                    for dt in range(NT):
                        # the weight tile is deliberately NOT in the fence:
                        # each matmul carries its own single DMA wait, so
                        # the stream starts as soon as x + weights land.
                        deps = [xt[:, : NT * C // 4]]
                        if masked_rank1:
                            deps += [neg_col[:], mrow[:]]
                        ps = mm_psum(deps)
                        for mt in range(NT):
                            ws = slice(mt * DH + dt * P, mt * DH + (dt + 1) * P)
                            for c0, c1 in chunks:
                                nc.tensor.matmul(
                                    ps[:, c0:c1],
                                    wta[:, ws],
                                    xt[:, mt * C + c0 : mt * C + c1],
                                    start=(mt == 0),
                                    stop=(mt == NT - 1) and not masked_rank1,
                                )
                        if masked_rank1:
                            for c0, c1 in chunks:
                                nc.tensor.matmul(
                                    ps[:, c0:c1], neg_col[:], mrow[:, c0:c1],
                                    start=False, stop=(c1 == chunks[-1][1]),
                                )
                        tiles.append(ps)
                    return tiles

                # K projection (perm feature order): rank-1 -1e9*pad row
                # forces phi_k at tail cols to 0
                next_hook()
                kt = []
                ksum = spool.tile([P, NT + 1], bf16, tag="ksum")
                for dt, ps in enumerate(project(W_K, True)):
                    bcol = bias_sb[:, BIAS_COL[W_K] + dt : BIAS_COL[W_K] + dt + 1]
                    E = apool.tile([P, C], f32, tag="E")
                    nc.scalar.activation(E[:], ps[:], Act.Exp, bias=bcol)
                    R = rrpool.tile([P, C], f32, tag="R")
                    nc.scalar.activation(R[:], ps[:], Act.Relu, bias=bcol)
                    t = kvqpool.tile([P, C], bf16, tag="kt")
                    nc.vector.scalar_tensor_tensor(
                        out=t[:], in0=E[:], scalar=1.0, in1=R[:],
                        op0=Op.min, op1=Op.add,
                        accum_out=ksum[:, dt : dt + 1],
                    )
                    kt.append(t)

                # ---- pad-mask keep tile (1 - pad broadcast): emitted here,
                # after the K projection, so the PE's first work needs only
                # x + K weights ----
                kb_ps = mm_psum([ones_col[:], mrow[:]])
                for c0, c1 in chunks:
                    nc.tensor.matmul(
                        kb_ps[:, c0:c1], ones_col[:], mrow[:, c0:c1],
                        start=True, stop=True,
                    )
                keep_tile = ktpool.tile([P, C], f32, tag="keeptile")
                nc.vector.tensor_scalar(
                    out=keep_tile[:], in0=kb_ps[:], scalar1=-1.0, scalar2=1.0,
                    op0=Op.mult, op1=Op.add,
                )

                # V projection: (psum + bv) * keep  (zeroes tail cols)
                next_hook()
                vt = []
                for dt, ps in enumerate(project(W_V, False)):
                    bcol = bias_sb[:, BIAS_COL[W_V] + dt : BIAS_COL[W_V] + dt + 1]
                    t = kvqpool.tile([P, C], bf16, tag="vt")
                    nc.vector.scalar_tensor_tensor(
                        out=t[:], in0=ps[:], scalar=bcol, in1=keep_tile[:],
                        op0=Op.add, op1=Op.mult,
                    )
                    vt.append(t)

                # Q projection: unmasked phi_q (tail s cols discarded on host)
                next_hook()
                qt = []
                for dt, ps in enumerate(project(W_Q, False)):
                    bcol = bias_sb[:, BIAS_COL[W_Q] + dt : BIAS_COL[W_Q] + dt + 1]
                    E = apool.tile([P, C], f32, tag="E")
                    nc.scalar.activation(E[:], ps[:], Act.Exp, bias=bcol)
                    R = rrpool.tile([P, C], f32, tag="R")
                    nc.scalar.activation(R[:], ps[:], Act.Relu, bias=bcol)
                    t = kvqpool.tile([P, C], bf16, tag="qt")
                    nc.vector.scalar_tensor_tensor(
                        out=t[:], in0=E[:], scalar=1.0, in1=R[:],
                        op0=Op.min, op1=Op.add,
                        # the den matmuls run at N=2 with a pad column of
                        # ksum that must hold real data — fill it with a
                        # q-side accum.
                        accum_out=(
                            ksum[:, NT : NT + 1] if dt == NT - 1 else None
                        ),
                    )
                    qt.append(t)

                # ---- A = V @ phi_k^T  (A[i,j], i=v col (compact), j=k col) ----
                next_hook()
                at = []
                for it in range(NTC):
                    ri = rows[it]
                    isl = slice(it * P, it * P + ri)
                    ps = mm_psum([t[:] for t in vt] + [t[:] for t in kt])
                    for dt in range(NT):
                        for c0, c1 in chunks:
                            nc.tensor.matmul(
                                ps[:ri, c0:c1],
                                vt[dt][:, isl],
                                kt[dt][:, c0:c1],
                                start=(dt == 0), stop=(dt == NT - 1),
                            )
                    t = atpool.tile([P, C], bf16, tag="at")
                    nc.vector.tensor_copy(t[:ri, :], ps[:ri, :])
                    at.append(t)

                # ---- denominator (only needs qt + ksum; runs while the at
                # evacuations drain so the O loop ships output immediately) ----
                next_hook()
                zs = []
                for st in range(NTC):
                    rs = rows[st]
                    ss = slice(st * P, st * P + rs)
                    dps = dpool.tile([P, 2], f32, tag="den")
                    fence([t[:] for t in qt] + [ksum[:]], [dps[:]])
                    for dt in range(NT):
                        nc.tensor.matmul(
                            dps[:rs, :],
                            qt[dt][:, ss],
                            ksum[:, dt : dt + 2],
                            start=(dt == 0), stop=(dt == NT - 1),
                        )
                    dsb = spool.tile([P, 1], f32, tag="dsb")
                    nc.vector.tensor_scalar(
                        out=dsb[:rs], in0=dps[:rs, 0:1], scalar1=float(EPS),
                        scalar2=None, op0=Op.max,
                    )
                    z = spool.tile([P, 1], f32, tag="z", bufs=NTC + 1)
                    nc.vector.reciprocal(z[:rs], dsb[:rs])
                    zs.append(z)

                # ---- O = phi_q[:, :C] @ A, scale, single batched store ----
                next_hook()
                o = opool.tile([P, NTC * C], bf16, tag="ost")
                for st in range(NTC):
                    rs = rows[st]
                    ss = slice(st * P, st * P + rs)
                    ps = pspool.tile([P, C], f32, tag="mm")
                    fence([t[:] for t in qt] + [t[:] for t in at], [ps[:]])
                    for it in range(NTC):
                        ri = rows[it]
                        for c0, c1 in chunks:
                            nc.tensor.matmul(
                                ps[:rs, c0:c1],
                                qt[it][:ri, ss],
                                at[it][:ri, c0:c1],
                                start=(it == 0), stop=(it == NTC - 1),
                            )
                    nc.vector.tensor_scalar(
                        out=o[:rs, st * C : (st + 1) * C], in0=ps[:rs, :],
                        scalar1=zs[st][:rs], scalar2=None, op0=Op.mult,
                    )
                    # ship full-row blocks as they complete so the store
                    # overlaps the remaining O groups
                    if rs == P and (st % 2 == 1 or st == NTC - 1):
                        lo = (st // 2) * 2 * C
                        nc.sync.dma_start(
                            out_ext[b, :, lo : (st + 1) * C],
                            o[:, lo : (st + 1) * C],
                        )
                    elif rs < P:
                        if st % 2 == 1:
                            lo = (st // 2) * 2 * C
                            nc.sync.dma_start(
                                out_ext[b, :, lo : st * C], o[:, lo : st * C]
                            )
                        nc.sync.dma_start(
                            out_ext[b, :rs, st * C : (st + 1) * C],
                            o[:rs, st * C : (st + 1) * C],
                        )

    nc.compile()
    return nc


def _prepare_in_maps(inputs):
    import concourse.mybir as mybir

    npbf16 = mybir.dt.np(mybir.dt.bfloat16)

    x = np.asarray(inputs["x"], np.float32)
    pm = np.asarray(inputs["padding_mask"])
    W = [np.asarray(inputs[k], np.float32) for k in ("Wq", "Wk", "Wv")]
    bias = [np.asarray(inputs[k], np.float32) for k in ("bq", "bk", "bv")]

    idx_list = [np.nonzero(pm[b] != 1)[0] for b in range(B)]
    ns = [len(i) for i in idx_list]
    C = max(max(ns), 2)

    # partition-major device layouts (see _build_nc)
    xt = np.zeros((B, P, NT, C), npbf16)
    wt = np.zeros((B, 3, P, NT, DH), npbf16)
    bias_t = np.zeros((B, P, 3 * NT), np.float32)
    mrow = np.zeros((B, C), npbf16)
    for b in range(B):
        idx = idx_list[b]
        n = ns[b]
        rest = np.nonzero(pm[b] == 1)[0]
        perm = np.concatenate([idx, rest])
        # [p, mt, c] = x[idx[c], mt*P+p]
        xc = x[b, idx, :].T.reshape(NT, P, n).transpose(1, 0, 2)
        xt[b, :, :, :n] = xc.astype(npbf16)
        mrow[b, n:] = 1.0
        for w in range(3):
            wp = W[w][perm]  # [DH(d' perm), DM(m)]
            # [p, mt, d] = wp[d, mt*P+p]
            wt[b, w] = wp.T.reshape(NT, P, DH).transpose(1, 0, 2).astype(npbf16)
            bias_t[b, :, w * NT : (w + 1) * NT] = bias[w][perm].reshape(NT, P).T

    consts = np.stack(
        [np.ones(P, np.float32), np.full(P, NEG, np.float32)]
    ).astype(npbf16)

    in_maps = []
    for i in range(NCORES):
        sl = slice(BPC * i, BPC * (i + 1))
        in_maps.append(
            {
                "xt": np.ascontiguousarray(xt[sl]).reshape(BPC, P, NT * C),
                "wt": np.ascontiguousarray(wt[sl]).reshape(BPC, 3, P, NT * DH),
                "bias": np.ascontiguousarray(bias_t[sl]),
                "mrow": np.ascontiguousarray(mrow[sl]),
                "consts": consts,
            }
        )
    return C, ns, idx_list, in_maps


def _run(inputs, **kw):
    from concourse.bass_utils import run_bass_kernel_spmd

    C, ns, idx_list, in_maps = _prepare_in_maps(inputs)
    if C not in _CACHE:
        _CACHE[C] = _build_nc(C)
    nc = _CACHE[C]
    res = run_bass_kernel_spmd(nc, in_maps, core_ids=list(range(NCORES)), **kw)
    NTC = (C + P - 1) // P
    out = np.zeros((B, S, S), np.float32)
    for b in range(B):
        core, off = divmod(b, BPC)
        n = ns[b]
        idx = idx_list[b]
        # [P, NTC, C] -> [NTC*P, C] row-major over s
        oc = (
            np.asarray(res.results[core]["out"])[off]
            .reshape(P, NTC, C)
            .transpose(1, 0, 2)
            .reshape(NTC * P, C)
            .astype(np.float32)
        )
        out[b][np.ix_(idx, idx)] = oc[:n, :n]
    return out, res


def kernel(**inputs):
    out, _ = _run(inputs)
    return out
